# revision 1
# baseline (speedup 1.0000x reference)
"""Trainium2 Bass kernel: fused attention block (QKV proj -> MHA -> out proj).

Reference (per batch item b, NUM_HEADS=12, Dh=64):
    qkv = x @ W_qkv; q,k,v per head
    attn = softmax(q @ k^T / 8) @ v
    out  = concat_heads(attn) @ W_proj + b_proj

Sharding: data-parallel over batch across 8 NeuronCores (128 batch items
per core), weights replicated. One SPMD Bass program, per-core inputs.

Per-core plan (128 batches, groups of G=8 batches = 392 tokens).
All matmuls use float32r (full PE rate, ~1.6e-4 rel err; requires even
moving dim N and even psum column offsets -> 50-wide per-batch slots).

  A. DMA x token-major, PE-transpose to feature-major xT
  B. q,k GEMM feature-major: psum[128co, T] = Wqkv_tile.T @ xT.
     q co-tiles stored naturally [128, T+2]; k co-tiles scattered into
     block-diagonal form kbd[j]: per batch a [128, 98] block with
     k_h(2j) in rows 0:64 cols 0:49 and k_h(2j+1) in rows 64:128
     cols 49:98 (zeros elsewhere, pre-loaded once from a host constant).
  C. v GEMM token-major -> scratch; SBUF->SBUF DMA scatter into vbd[j]:
     per batch a [98, 128] block with v_h(2j) rows 0:49 cols 0:64 and
     v_h(2j+1) rows 49:98 cols 64:128 (zeros preloaded once).
  D. attention per (head-pair j, batch): both heads in one matmul chain:
       sT2 = kbd_b.T @ q_pair          [98, 50]  scores, heads stacked
       eT2 = exp(sT2/8)                ACT, one op per pair-cell [98,400]
       r2  = onesbd.T @ eT2            [2, 400]  row sums per head
       po  = vbd_b.T @ eT2             [128, 50] unnorm out^T, both heads
     r rows gathered (ACT copy + SBUF-shift DMA) into rgrp[12, T];
     one batched reciprocal per group; per j: broadcast matmul
     (sel2 selector) -> bc[128, T]; unT[j] *= bc (in-place DVE).
  E. proj GEMM token-major: psum[tok, 384] = unT_toktile.T @ Wproj + bias
     -> DMA out (contiguous rows)
"""
import sys

sys.path.insert(0, "/opt/trn_rl_repo")

import numpy as np

NUM_CORES = 8
B_CORE = 128          # batch items per core
SEQ = 49              # tokens per batch item
C = 768               # channels
H = 12                # heads
G = 8                 # batch items per group
T = SEQ * G           # 392 tokens per group (even)
TP = T + 2            # padded q tile width
TOK = B_CORE * SEQ    # 6272 tokens per core
N_GROUPS = B_CORE // G
KBD_W = G * 98        # kbd block row width per j
VBD_W = G * 128       # vbd block row width per j

_CACHE = {}


def _consts():
    ones1 = np.ones((1, 128), dtype=np.float32)
    ident = np.eye(128, dtype=np.float32)
    sel2 = np.zeros((H, 6 * 128), dtype=np.float32)
    for j in range(6):
        sel2[2 * j, 128 * j:128 * j + 64] = 1.0
        sel2[2 * j + 1, 128 * j + 64:128 * (j + 1)] = 1.0
    onesbd = np.zeros((98, 2), dtype=np.float32)
    onesbd[0:49, 0] = 1.0
    onesbd[49:98, 1] = 1.0
    kbdz = np.zeros((128, 6 * KBD_W), dtype=np.float32)
    vbdz = np.zeros((98, 6 * VBD_W), dtype=np.float32)
    return {"ones1": ones1, "ident": ident, "sel2": sel2,
            "onesbd": onesbd, "kbdz": kbdz, "vbdz": vbdz}


def _build():
    import concourse.bacc as bacc
    import concourse.mybir as mybir
    import concourse.tile as tile

    F32 = mybir.dt.float32
    F32R = mybir.dt.float32r
    EXP = mybir.ActivationFunctionType.Exp

    nc = bacc.Bacc("TRN2", target_bir_lowering=False)

    d_x = nc.declare_dram_parameter("x", [TOK, C], F32, isOutput=False)
    d_wqkv = nc.declare_dram_parameter("wqkv", [C, 3 * C], F32R, isOutput=False)
    d_wproj = nc.declare_dram_parameter("wproj", [C, C], F32R, isOutput=False)
    d_bias = nc.declare_dram_parameter("bias", [1, C], F32R, isOutput=False)
    d_ones1 = nc.declare_dram_parameter("ones1", [1, 128], F32R, isOutput=False)
    d_ident = nc.declare_dram_parameter("ident", [128, 128], F32, isOutput=False)
    d_sel2 = nc.declare_dram_parameter("sel2", [H, 6 * 128], F32R, isOutput=False)
    d_onesbd = nc.declare_dram_parameter("onesbd", [98, 2], F32R, isOutput=False)
    d_kbdz = nc.declare_dram_parameter("kbdz", [128, 6 * KBD_W], F32R,
                                       isOutput=False)
    d_vbdz = nc.declare_dram_parameter("vbdz", [98, 6 * VBD_W], F32R,
                                       isOutput=False)
    d_out = nc.declare_dram_parameter("out", [TOK, C], F32, isOutput=True)

    # token tiles within a group
    tts = []
    o = 0
    while o < T:
        tts.append((o, min(128, T - o)))
        o += 128

    with tile.TileContext(nc) as tc, \
         nc.allow_low_precision(reason="float32r storage for full-rate matmul"):
        with tc.tile_pool(name="wres", bufs=1) as wres, \
             tc.tile_pool(name="xtm", bufs=4) as p_xtm, \
             tc.tile_pool(name="xT", bufs=1) as p_xT, \
             tc.tile_pool(name="qk", bufs=1) as p_qk, \
             tc.tile_pool(name="vscr", bufs=2) as p_vscr, \
             tc.tile_pool(name="eT", bufs=3) as p_eT, \
             tc.tile_pool(name="rr", bufs=1) as p_rr, \
             tc.tile_pool(name="bc", bufs=2) as p_bc, \
             tc.tile_pool(name="unT", bufs=1) as p_unT, \
             tc.tile_pool(name="osb", bufs=2) as p_osb, \
             tc.tile_pool(name="psA", bufs=2, space="PSUM") as psA, \
             tc.tile_pool(name="psB", bufs=2, space="PSUM") as psB, \
             tc.tile_pool(name="psS", bufs=2, space="PSUM") as psS, \
             tc.tile_pool(name="psO", bufs=2, space="PSUM") as psO:

            # ---- resident weights / constants ----
            w_qkv = []
            for ci in range(6):
                t = wres.tile([128, 3 * C], F32R, tag=f"wqkv{ci}")
                nc.sync.dma_start(t[:], d_wqkv[128 * ci:128 * (ci + 1), :])
                w_qkv.append(t)
            w_proj = []
            for ci in range(6):
                t = wres.tile([128, C], F32R, tag=f"wproj{ci}")
                nc.sync.dma_start(t[:], d_wproj[128 * ci:128 * (ci + 1), :])
                w_proj.append(t)
            ones1 = wres.tile([1, 128], F32R, tag="ones1")
            nc.sync.dma_start(ones1[:], d_ones1[:])
            sel2 = wres.tile([H, 6 * 128], F32R, tag="sel2")
            nc.sync.dma_start(sel2[:], d_sel2[:])
            onesbd = wres.tile([98, 2], F32R, tag="onesbd")
            nc.sync.dma_start(onesbd[:], d_onesbd[:])
            ident = wres.tile([128, 128], F32, tag="ident")
            nc.sync.dma_start(ident[:], d_ident[:])
            kbd = wres.tile([128, 6 * KBD_W], F32R, tag="kbd")
            nc.sync.dma_start(kbd[:], d_kbdz[:])
            vbd = wres.tile([98, 6 * VBD_W], F32R, tag="vbd")
            nc.sync.dma_start(vbd[:], d_vbdz[:])
            bias_sb = wres.tile([1, C], F32R, tag="bias_sb")
            nc.sync.dma_start(bias_sb[:], d_bias[:])
            bias_bc = wres.tile([128, C], F32, tag="bias_bc")
            for half in range(2):
                pb = psB.tile([128, 384], F32, tag="psB")
                nc.tensor.matmul(pb[:], ones1[:],
                                 bias_sb[:, 384 * half:384 * (half + 1)],
                                 start=True, stop=True)
                nc.scalar.copy(bias_bc[:, 384 * half:384 * (half + 1)], pb[:])

            for g in range(N_GROUPS):
                r0 = g * T  # first token row of group

                # ---- A: load x token-major, transpose to xT ----
                x_tm = []
                for (to, tk) in tts:
                    t = p_xtm.tile([128, C], F32, tag="xtm")
                    nc.sync.dma_start(t[:tk, :], d_x[r0 + to:r0 + to + tk, :])
                    x_tm.append(t)
                xT = [p_xT.tile([128, T], F32R, tag=f"xT{ci}", name=f"xT{ci}")
                      for ci in range(6)]
                for tti, (to, tk) in enumerate(tts):
                    for ci in range(6):
                        pt = psB.tile([128, 384], F32, tag="psB")
                        nc.tensor.transpose(
                            pt[:, :tk],
                            x_tm[tti][:tk, 128 * ci:128 * (ci + 1)],
                            ident[:tk, :tk])
                        nc.vector.tensor_copy(xT[ci][:, to:to + tk], pt[:, :tk])

                # ---- B: q,k GEMM; q natural, k scattered block-diag ----
                qk = []
                for j in range(12):
                    pq = psA.tile([128, TP], F32, tag="psA")
                    for ci in range(6):
                        nc.tensor.matmul(
                            pq[:, :T],
                            w_qkv[ci][:, 128 * j:128 * (j + 1)],
                            xT[ci][:, :T],
                            start=(ci == 0), stop=(ci == 5))
                    if j < 6:
                        t = p_qk.tile([128, TP], F32R, tag=f"q{j}", name=f"q{j}")
                        nc.vector.tensor_copy(t[:, :T], pq[:, :T])
                        nc.vector.tensor_copy(t[:, T:T + 2], pq[:, :2])
                        qk.append(t)
                    else:
                        jj = j - 6
                        kv = kbd[:, jj * KBD_W:(jj + 1) * KBD_W].rearrange(
                            "p (b c) -> p b c", c=98)
                        nc.vector.tensor_copy(
                            kv[0:64, :, 0:49],
                            pq[0:64, :T].rearrange("p (b c) -> p b c", c=49))
                        nc.vector.tensor_copy(
                            kv[64:128, :, 49:98],
                            pq[64:128, :T].rearrange("p (b c) -> p b c", c=49))

                # ---- C: v GEMM token-major + block-diag scatter ----
                v4 = vbd.rearrange("p (j b c) -> p j b c", b=G, c=128)
                for tti, (to, tk) in enumerate(tts):
                    scr = p_vscr.tile([128, C], F32R, tag="vscr")
                    for half in range(2):
                        pv = psB.tile([128, 384], F32, tag="psB")
                        for ci in range(6):
                            nc.tensor.matmul(
                                pv[:tk, :],
                                xT[ci][:, to:to + tk],
                                w_qkv[ci][:, 1536 + 384 * half:
                                           1536 + 384 * (half + 1)],
                                start=(ci == 0), stop=(ci == 5))
                        nc.vector.tensor_copy(
                            scr[:tk, 384 * half:384 * (half + 1)], pv[:tk, :])
                    # scatter batch segments of this token tile
                    for b in range(G):
                        lo = max(b * SEQ, to)
                        hi = min((b + 1) * SEQ, to + tk)
                        if lo >= hi:
                            continue
                        sl, sh = lo - b * SEQ, hi - b * SEQ  # rows in block
                        src = scr[lo - to:hi - to, :]
                        # even heads -> rows sl:sh, cols 0:64 of block
                        nc.sync.dma_start(
                            v4[sl:sh, :, b, 0:64],
                            src.rearrange("p (j two c) -> p j two c",
                                          two=2, c=64)[:, :, 0, :])
                        # odd heads -> rows 49+sl:49+sh, cols 64:128
                        nc.sync.dma_start(
                            v4[49 + sl:49 + sh, :, b, 64:128],
                            src.rearrange("p (j two c) -> p j two c",
                                          two=2, c=64)[:, :, 1, :])

                # ---- D: attention, one cell per head pair ----
                unT = [p_unT.tile([128, T], F32R, tag=f"unT{ci}", name=f"unT{ci}")
                       for ci in range(6)]
                rgrp = p_rr.tile([H, T], F32, tag="rgrp")
                for j in range(6):
                    ps = psS.tile([98, 50 * G], F32, tag="psS")
                    for b in range(G):
                        nc.tensor.matmul(
                            ps[:, 50 * b:50 * b + 50],
                            kbd[:, j * KBD_W + 98 * b:j * KBD_W + 98 * b + 98],
                            qk[j][:, 49 * b:49 * b + 50],
                            start=True, stop=True)
                    eT = p_eT.tile([98, 50 * G], F32R, tag="eT")
                    nc.scalar.activation(eT[:], ps[:], EXP, scale=0.125)
                    pr = psB.tile([2, 50 * G], F32, tag="psB")
                    nc.tensor.matmul(pr[:], onesbd[:], eT[:],
                                     start=True, stop=True)
                    po = psO.tile([128, 50 * G], F32, tag="psO")
                    for b in range(G):
                        nc.tensor.matmul(
                            po[:, 50 * b:50 * b + 50],
                            vbd[:, j * VBD_W + 128 * b:j * VBD_W + 128 * (b + 1)],
                            eT[:, 50 * b:50 * b + 50],
                            start=True, stop=True)
                    nc.vector.tensor_copy(
                        unT[j][:, :].rearrange("p (b c) -> p b c", c=49),
                        po[:, :].rearrange("p (b c) -> p b c", c=50)[:, :, 0:49])
                    r2 = p_bc.tile([2, T], F32, tag="r2")
                    nc.scalar.copy(
                        r2.rearrange("p (b c) -> p b c", c=49),
                        pr.rearrange("p (b c) -> p b c", c=50)[:, :, 0:49])
                    nc.sync.dma_start(rgrp[2 * j:2 * j + 2, :], r2[:])
                # one batched reciprocal for all 12 heads of the group
                rr = p_rr.tile([H, T], F32R, tag="rr")
                nc.vector.reciprocal(rr[:], rgrp[:])
                for j in range(6):
                    pbc = psA.tile([128, TP], F32, tag="psA")
                    nc.tensor.matmul(pbc[:, :T], sel2[:, 128 * j:128 * (j + 1)],
                                     rr[:], start=True, stop=True)
                    bc = p_bc.tile([128, T], F32, tag="bc")
                    nc.scalar.copy(bc[:], pbc[:, :T])
                    nc.vector.tensor_mul(out=unT[j][:, :], in0=unT[j][:, :],
                                         in1=bc[:])

                # ---- E: proj GEMM + bias -> out ----
                for tti, (to, tk) in enumerate(tts):
                    osb = p_osb.tile([128, C], F32, tag="osb")
                    for half in range(2):
                        pp = psB.tile([128, 384], F32, tag="psB")
                        for ci in range(6):
                            nc.tensor.matmul(
                                pp[:tk, :],
                                unT[ci][:, to:to + tk],
                                w_proj[ci][:, 384 * half:384 * (half + 1)],
                                start=(ci == 0), stop=(ci == 5))
                        nc.vector.tensor_add(
                            out=osb[:tk, 384 * half:384 * (half + 1)],
                            in0=pp[:tk, :],
                            in1=bias_bc[:tk, 384 * half:384 * (half + 1)])
                    nc.sync.dma_start(d_out[r0 + to:r0 + to + tk, :],
                                      osb[:tk, :])

    nc.compile()
    return nc


def kernel(x, W_qkv, W_proj, b_proj):
    from concourse.bass_utils import run_bass_kernel_spmd

    if "nc" not in _CACHE:
        _CACHE["nc"] = _build()
    nc = _CACHE["nc"]

    x = np.ascontiguousarray(np.asarray(x, dtype=np.float32))
    B, N, Cc = x.shape
    consts = _consts()
    W_qkv = np.ascontiguousarray(np.asarray(W_qkv, dtype=np.float32))
    W_proj = np.ascontiguousarray(np.asarray(W_proj, dtype=np.float32))
    bias = np.ascontiguousarray(
        np.asarray(b_proj, dtype=np.float32).reshape(1, Cc))

    in_maps = []
    for i in range(NUM_CORES):
        m = {"x": np.ascontiguousarray(
                x[i * B_CORE:(i + 1) * B_CORE].reshape(TOK, Cc)),
             "wqkv": W_qkv, "wproj": W_proj, "bias": bias}
        m.update(consts)
        in_maps.append(m)
    res = run_bass_kernel_spmd(nc, in_maps, list(range(NUM_CORES)))
    out = np.empty((B, N, Cc), dtype=np.float32)
    for i in range(NUM_CORES):
        out[i * B_CORE:(i + 1) * B_CORE] = res.results[i]["out"].reshape(
            B_CORE, N, Cc)
    return out



# revision 4
# speedup vs baseline: 1.4720x; 1.4720x over previous
"""Trainium2 Bass kernel: fused attention block (QKV proj -> MHA -> out proj).

Reference (per batch item b, NUM_HEADS=12, Dh=64):
    qkv = x @ W_qkv; q,k,v per head
    attn = softmax(q @ k^T / 8) @ v
    out  = concat_heads(attn) @ W_proj + b_proj

Sharding: data-parallel over batch across 8 NeuronCores (128 batch items
per core), weights replicated. One SPMD Bass program, per-core inputs.

Per-core plan (128 batches, groups of G=8 batches = 392 tokens). All
matmul operands bf16 (fp32 accumulate in PSUM). x is pre-transposed +
cast on host to feature-major xT[768, 6272] per core, so no on-device
transposes. Software-pipelined one group deep: attention+proj of group
g-1 is emitted alongside QKV of group g so the PE never idles.

  B. q,k GEMM feature-major: psum[128co, T] = Wqkv_tile.T @ xT.
     q co-tiles stored naturally [128, T+2]; k co-tiles DVE-scattered
     into block-diagonal kbd[g%2]: per (j, b) a [128, 128] block with
     k_h(2j) rows 0:64 cols 0:49 and k_h(2j+1) rows 64:128 cols 64:113
     (zeros elsewhere, memset once).
  C. v GEMM token-major over 98-token tiles -> scratch; 4 aligned
     SBUF->SBUF DMAs per tile scatter into vbd[g%2]: per (j, b) a
     [128, 128] block with v_h(2j) rows 0:49 cols 0:64 and v_h(2j+1)
     rows 64:113 cols 64:128.
  D. attention per (head-pair j, batch): both heads in one matmul chain:
       sT2 = kbd_jb.T @ q_pair        [128, 50]  scores, heads stacked
       eT2 = exp(sT2/8)               ACT, [128, 400] per j
       r2  = onesbd.T @ eT2           [2, 400]   row sums per head
       po  = vbd_jb.T @ eT2           [128, 50]  unnorm out^T, both heads
     r rows gathered (ACT copy + SBUF-shift DMA) into rgrp[12, T];
     one batched reciprocal per group; per j: broadcast matmul
     (sel2 selector, f32r) -> psum bc[128, T]; unT[j] *= bc (DVE).
  E. proj GEMM token-major: psum[98tok, 384] = unT_slice.T @ Wproj,
     bias accumulated via ones-row matmul -> copy -> DMA out.
"""
import sys

sys.path.insert(0, "/opt/trn_rl_repo")

import numpy as np

NUM_CORES = 8
B_CORE = 128          # batch items per core
SEQ = 49              # tokens per batch item
C = 768               # channels
H = 12                # heads
G = 8                 # batch items per group
T = SEQ * G           # 392 tokens per group (even)
TP = T + 2            # padded q tile width
TOK = B_CORE * SEQ    # 6272 tokens per core
N_GROUPS = B_CORE // G
BDW = G * 128         # kbd/vbd block row width per j (padded 128/batch)

_CACHE = {}


def _consts():
    import ml_dtypes
    bf16 = ml_dtypes.bfloat16
    ones1 = np.ones((1, 128), dtype=bf16)
    # sel2: broadcast head sums over Dh partitions (f32r matmul)
    sel2 = np.zeros((H, 6 * 128), dtype=np.float32)
    for j in range(6):
        sel2[2 * j, 128 * j:128 * j + 64] = 1.0
        sel2[2 * j + 1, 128 * j + 64:128 * (j + 1)] = 1.0
    # onesbd: column sums of eT rows per head (keys at rows 0:49, 64:113)
    onesbd = np.zeros((128, 2), dtype=bf16)
    onesbd[0:49, 0] = 1.0
    onesbd[64:113, 1] = 1.0
    return {"ones1": ones1, "sel2": sel2, "onesbd": onesbd}


def _build():
    import concourse.bacc as bacc
    import concourse.mybir as mybir
    import concourse.tile as tile

    F32 = mybir.dt.float32
    F32R = mybir.dt.float32r
    BF16 = mybir.dt.bfloat16
    EXP = mybir.ActivationFunctionType.Exp

    nc = bacc.Bacc("TRN2", target_bir_lowering=False)

    d_xt = nc.declare_dram_parameter("xt", [C, TOK], BF16, isOutput=False)
    d_wqkv = nc.declare_dram_parameter("wqkv", [C, 3 * C], BF16, isOutput=False)
    d_wproj = nc.declare_dram_parameter("wproj", [C, C], BF16, isOutput=False)
    d_bias = nc.declare_dram_parameter("bias", [1, C], BF16, isOutput=False)
    d_ones1 = nc.declare_dram_parameter("ones1", [1, 128], BF16, isOutput=False)
    d_sel2 = nc.declare_dram_parameter("sel2", [H, 6 * 128], F32R,
                                       isOutput=False)
    d_onesbd = nc.declare_dram_parameter("onesbd", [128, 2], BF16,
                                         isOutput=False)
    d_out = nc.declare_dram_parameter("out", [TOK, C], F32, isOutput=True)

    # 98-token tiles within a group (4 per group, batch-pair aligned)
    tts = [(o, 98) for o in range(0, T, 98)]

    with tile.TileContext(nc) as tc, \
         nc.allow_low_precision(reason="bf16 operands, fp32 accumulate"):
        with tc.tile_pool(name="wres", bufs=1) as wres, \
             tc.tile_pool(name="xT", bufs=2) as p_xT, \
             tc.tile_pool(name="qk", bufs=2) as p_qk, \
             tc.tile_pool(name="vscr", bufs=2) as p_vscr, \
             tc.tile_pool(name="eT", bufs=2) as p_eT, \
             tc.tile_pool(name="rr", bufs=2) as p_rr, \
             tc.tile_pool(name="unT", bufs=2) as p_unT, \
             tc.tile_pool(name="osb", bufs=2) as p_osb, \
             tc.tile_pool(name="psA", bufs=2, space="PSUM") as psA, \
             tc.tile_pool(name="psB", bufs=2, space="PSUM") as psB, \
             tc.tile_pool(name="psS", bufs=2, space="PSUM") as psS, \
             tc.tile_pool(name="psO", bufs=2, space="PSUM") as psO:

            # ---- resident weights / constants ----
            w_qkv = []
            for ci in range(6):
                t = wres.tile([128, 3 * C], BF16, tag=f"wqkv{ci}")
                nc.sync.dma_start(t[:], d_wqkv[128 * ci:128 * (ci + 1), :])
                w_qkv.append(t)
            w_proj = []
            for ci in range(6):
                t = wres.tile([128, C], BF16, tag=f"wproj{ci}")
                nc.sync.dma_start(t[:], d_wproj[128 * ci:128 * (ci + 1), :])
                w_proj.append(t)
            ones1 = wres.tile([1, 128], BF16, tag="ones1")
            nc.sync.dma_start(ones1[:], d_ones1[:])
            sel2 = wres.tile([H, 6 * 128], F32R, tag="sel2")
            nc.sync.dma_start(sel2[:], d_sel2[:])
            onesbd = wres.tile([128, 2], BF16, tag="onesbd")
            nc.sync.dma_start(onesbd[:], d_onesbd[:])
            bias_sb = wres.tile([1, C], BF16, tag="bias_sb")
            nc.sync.dma_start(bias_sb[:], d_bias[:])
            # double-buffered block-diagonal k/v, zeroed once
            kbds, vbds = [], []
            for s in range(2):
                kb = wres.tile([128, 6 * BDW], BF16, tag=f"kbd{s}")
                nc.vector.memset(kb[:], 0.0)
                kbds.append(kb)
                vb = wres.tile([128, 6 * BDW], BF16, tag=f"vbd{s}")
                nc.vector.memset(vb[:], 0.0)
                vbds.append(vb)

            state = {}

            def emit_qkv(g):
                """Phases B+C for group g: load xT, qk GEMM, v GEMM."""
                r0 = g * T
                kbd, vbd = kbds[g % 2], vbds[g % 2]
                xT = p_xT.tile([128, 6 * T], BF16, tag="xT", name="xT")
                xv = xT[:].rearrange("p (ci t) -> p ci t", t=T)
                nc.sync.dma_start(
                    xv,
                    d_xt[:].rearrange("(ci p) t -> p ci t", ci=6)
                        [:, :, r0:r0 + T])

                # q,k GEMM feature-major; q natural, k scattered block-diag
                qk = []
                for j in range(12):
                    pq = psA.tile([128, TP], F32, tag="psA", name="psa")
                    for ci in range(6):
                        nc.tensor.matmul(
                            pq[:, :T],
                            w_qkv[ci][:, 128 * j:128 * (j + 1)],
                            xv[:, ci, :],
                            start=(ci == 0), stop=(ci == 5))
                    if j < 6:
                        t = p_qk.tile([128, TP], BF16, tag=f"q{j}", name=f"q{j}")
                        nc.vector.tensor_copy(t[:, :T], pq[:, :T])
                        nc.vector.tensor_copy(t[:, T:T + 2], pq[:, :2])
                        qk.append(t)
                    else:
                        jj = j - 6
                        kv = kbd[:, jj * BDW:(jj + 1) * BDW].rearrange(
                            "p (b two t) -> p b two t", two=2, t=64)
                        nc.vector.tensor_copy(
                            kv[0:64, :, 0, 0:49],
                            pq[0:64, :T].rearrange("p (b t) -> p b t", t=49))
                        nc.vector.tensor_copy(
                            kv[64:128, :, 1, 0:49],
                            pq[64:128, :T].rearrange("p (b t) -> p b t", t=49))
                state[("qk", g)] = qk

                # v GEMM token-major over 98-token tiles + scatter
                v5 = vbd[:].rearrange("p (j b two c) -> p j b two c",
                                      b=G, two=2, c=64)
                for tti, (to, tk) in enumerate(tts):
                    scr = p_vscr.tile([98, C], BF16, tag="vscr", name="vscr")
                    for half in range(2):
                        pv = psB.tile([98, 384], F32, tag="psB", name="psb")
                        for ci in range(6):
                            nc.tensor.matmul(
                                pv[:, :],
                                xv[:, ci, to:to + tk],
                                w_qkv[ci][:, 1536 + 384 * half:
                                          1536 + 384 * (half + 1)],
                                start=(ci == 0), stop=(ci == 5))
                        nc.vector.tensor_copy(
                            scr[:, 384 * half:384 * (half + 1)], pv[:, :])
                    sv = scr[:].rearrange("p (j two c) -> p j two c",
                                          two=2, c=64)
                    for bl in range(2):
                        b = 2 * tti + bl
                        for two in range(2):
                            nc.sync.dma_start(
                                v5[64 * two:64 * two + 49, :, b, two, :],
                                sv[49 * bl:49 * bl + 49, :, two, :])

            def emit_attn(g):
                """Phases D+E for group g: attention, normalize, proj."""
                r0 = g * T
                kbd, vbd = kbds[g % 2], vbds[g % 2]
                qk = state.pop(("qk", g))

                unT = [p_unT.tile([128, T], BF16, tag=f"unT{ci}", name=f"unT{ci}")
                       for ci in range(6)]
                rgrp = p_rr.tile([H, T], F32, tag="rgrp", name="rgrp")
                for j in range(6):
                    ps = psS.tile([128, 50 * G], F32, tag="psS", name="ps")
                    for b in range(G):
                        nc.tensor.matmul(
                            ps[:, 50 * b:50 * b + 50],
                            kbd[:, j * BDW + 128 * b:j * BDW + 128 * (b + 1)],
                            qk[j][:, 49 * b:49 * b + 50],
                            start=True, stop=True)
                    eT = p_eT.tile([128, 50 * G], BF16, tag="eT", name="eT")
                    nc.scalar.activation(eT[:], ps[:], EXP, scale=0.125)
                    pr = psB.tile([2, 50 * G], F32, tag="psB", name="pr")
                    nc.tensor.matmul(pr[:], onesbd[:], eT[:],
                                     start=True, stop=True)
                    po = psO.tile([128, 50 * G], F32, tag="psO", name="po")
                    for b in range(G):
                        nc.tensor.matmul(
                            po[:, 50 * b:50 * b + 50],
                            vbd[:, j * BDW + 128 * b:j * BDW + 128 * (b + 1)],
                            eT[:, 50 * b:50 * b + 50],
                            start=True, stop=True)
                    nc.vector.tensor_copy(
                        unT[j][:, :].rearrange("p (b c) -> p b c", c=49),
                        po[:, :].rearrange("p (b c) -> p b c", c=50)[:, :, 0:49])
                    r2 = p_rr.tile([2, T], F32, tag="r2", name="r2")
                    nc.scalar.copy(
                        r2[:].rearrange("p (b c) -> p b c", c=49),
                        pr[:].rearrange("p (b c) -> p b c", c=50)[:, :, 0:49])
                    nc.sync.dma_start(rgrp[2 * j:2 * j + 2, :], r2[:])
                # one batched reciprocal for all 12 heads of the group
                rr = p_rr.tile([H, T], F32R, tag="rr", name="rr")
                nc.vector.reciprocal(rr[:], rgrp[:])
                for j in range(6):
                    pbc = psA.tile([128, TP], F32, tag="psA", name="psa")
                    nc.tensor.matmul(pbc[:, :T], sel2[:, 128 * j:128 * (j + 1)],
                                     rr[:], start=True, stop=True)
                    nc.vector.tensor_mul(out=unT[j][:, :], in0=unT[j][:, :],
                                         in1=pbc[:, :T])

                # proj GEMM + bias -> out
                for tti, (to, tk) in enumerate(tts):
                    osb = p_osb.tile([98, C], F32, tag="osb", name="osb")
                    for half in range(2):
                        pp = psB.tile([98, 384], F32, tag="psB", name="psb")
                        for ci in range(6):
                            nc.tensor.matmul(
                                pp[:, :],
                                unT[ci][:, to:to + tk],
                                w_proj[ci][:, 384 * half:384 * (half + 1)],
                                start=(ci == 0), stop=False)
                        nc.tensor.matmul(
                            pp[:, :], ones1[:, :tk],
                            bias_sb[:, 384 * half:384 * (half + 1)],
                            start=False, stop=True)
                        nc.vector.tensor_copy(
                            osb[:, 384 * half:384 * (half + 1)], pp[:, :])
                    nc.sync.dma_start(d_out[r0 + to:r0 + to + tk, :],
                                      osb[:, :])

            for step in range(N_GROUPS + 1):
                if step < N_GROUPS:
                    emit_qkv(step)
                if step >= 1:
                    emit_attn(step - 1)

    nc.compile()
    return nc


def make_in_maps(inputs):
    """Host-side prep: shard + transpose + cast. inputs keys as in
    reference.setup_inputs()."""
    import ml_dtypes
    bf16 = ml_dtypes.bfloat16

    x = np.asarray(inputs["x"], dtype=np.float32)
    B, N, Cc = x.shape
    W_qkv = np.ascontiguousarray(
        np.asarray(inputs["W_qkv"], dtype=np.float32).astype(bf16))
    W_proj = np.ascontiguousarray(
        np.asarray(inputs["W_proj"], dtype=np.float32).astype(bf16))
    bias = np.ascontiguousarray(
        np.asarray(inputs["b_proj"], dtype=np.float32)
        .reshape(1, Cc).astype(bf16))
    consts = _consts()

    in_maps = []
    for i in range(NUM_CORES):
        xt = np.ascontiguousarray(
            x[i * B_CORE:(i + 1) * B_CORE].reshape(TOK, Cc).T.astype(bf16))
        m = {"xt": xt, "wqkv": W_qkv, "wproj": W_proj, "bias": bias}
        m.update(consts)
        in_maps.append(m)
    return in_maps


def kernel(x, W_qkv, W_proj, b_proj):
    from concourse.bass_utils import run_bass_kernel_spmd

    if "nc" not in _CACHE:
        _CACHE["nc"] = _build()
    nc = _CACHE["nc"]

    in_maps = make_in_maps(
        {"x": x, "W_qkv": W_qkv, "W_proj": W_proj, "b_proj": b_proj})
    res = run_bass_kernel_spmd(nc, in_maps, list(range(NUM_CORES)))
    B, N, Cc = np.asarray(x).shape
    out = np.empty((B, N, Cc), dtype=np.float32)
    for i in range(NUM_CORES):
        out[i * B_CORE:(i + 1) * B_CORE] = res.results[i]["out"].reshape(
            B_CORE, N, Cc)
    return out


# revision 9
# speedup vs baseline: 2.0075x; 1.3638x over previous
"""Trainium2 Bass kernel: fused attention block (QKV proj -> MHA -> out proj).

Reference (per batch item b, NUM_HEADS=12, Dh=64):
    qkv = x @ W_qkv; q,k,v per head
    attn = softmax(q @ k^T / 8) @ v
    out  = concat_heads(attn) @ W_proj + b_proj

Sharding: data-parallel over batch across 8 NeuronCores (128 batch items
per core), weights replicated. One SPMD Bass program, per-core inputs.

Per-core plan (128 batches, groups of G=8 batches = 392 tokens). All
matmul operands bf16 (fp32 accumulate in PSUM). x is pre-transposed +
cast on host to feature-major xT[768, 6272] per core, so no on-device
transposes. Software-pipelined one group deep: attention+proj of group
g-1 is emitted alongside QKV of group g so the PE never idles.

  B. q,k GEMM feature-major: psum[128co, T] = Wqkv_tile.T @ xT.
     q co-tiles stored naturally [128, T+2]; k co-tiles DVE-scattered
     into block-diagonal kbd[g%2]: per (j, b) a [128, 128] block with
     k_h(2j) rows 0:64 cols 0:49 and k_h(2j+1) rows 64:128 cols 64:113
     (zeros elsewhere, memset once).
  C. v GEMM token-major over 98-token tiles -> scratch; 4 aligned
     SBUF->SBUF DMAs per tile scatter into vbd[g%2]: per (j, b) a
     [128, 128] block with v_h(2j) rows 0:49 cols 0:64 and v_h(2j+1)
     rows 64:113 cols 64:128.
  D. attention per (head-pair j, batch): both heads in one matmul chain:
       sT2 = kbd_jb.T @ q_pair        [128, 50]  scores, heads stacked
       eT2 = exp(sT2/8)               ACT, [128, 400] per j
       r2  = onesbd.T @ eT2           [2, 400]   row sums per head
       po  = vbd_jb.T @ eT2           [128, 50]  unnorm out^T, both heads
     r rows gathered (ACT copy + SBUF-shift DMA) into rgrp[12, T];
     one batched reciprocal per group; per j: broadcast matmul
     (sel2 selector, f32r) -> psum bc[128, T]; unT[j] *= bc (DVE).
  E. proj GEMM token-major: psum[98tok, 384] = unT_slice.T @ Wproj,
     bias accumulated via ones-row matmul -> copy -> DMA out.
"""
import sys

sys.path.insert(0, "/opt/trn_rl_repo")

import numpy as np

NUM_CORES = 8
B_CORE = 128          # batch items per core
SEQ = 49              # tokens per batch item
C = 768               # channels
H = 12                # heads
G = 8                 # batch items per group
T = SEQ * G           # 392 tokens per group (even)
TP = T + 2            # padded q tile width
TOK = B_CORE * SEQ    # 6272 tokens per core
N_GROUPS = B_CORE // G
BDW = G * 128         # kbd/vbd block row width per j (padded 128/batch)

_CACHE = {}


def _consts():
    import ml_dtypes
    bf16 = ml_dtypes.bfloat16
    ones1 = np.ones((1, 128), dtype=bf16)
    # sel2: broadcast head sums over Dh partitions (bf16 matmul)
    sel2 = np.zeros((H, 6 * 128), dtype=bf16)
    for j in range(6):
        sel2[2 * j, 128 * j:128 * j + 64] = 1.0
        sel2[2 * j + 1, 128 * j + 64:128 * (j + 1)] = 1.0
    # onesbd: column sums of eT rows per head (keys at rows 0:49, 64:113)
    onesbd = np.zeros((128, 2), dtype=bf16)
    onesbd[0:49, 0] = 1.0
    onesbd[64:113, 1] = 1.0
    return {"ones1": ones1, "sel2": sel2, "onesbd": onesbd}


def _build():
    import concourse.bacc as bacc
    import concourse.mybir as mybir
    import concourse.tile as tile

    F32 = mybir.dt.float32
    F32R = mybir.dt.float32r
    BF16 = mybir.dt.bfloat16
    EXP = mybir.ActivationFunctionType.Exp

    nc = bacc.Bacc("TRN2", target_bir_lowering=False)

    d_xt = nc.declare_dram_parameter("xt", [C, TOK], BF16, isOutput=False)
    d_wqkv = nc.declare_dram_parameter("wqkv", [C, 3 * C], BF16, isOutput=False)
    d_wproj = nc.declare_dram_parameter("wproj", [C, C], BF16, isOutput=False)
    d_bias = nc.declare_dram_parameter("bias", [1, C], BF16, isOutput=False)
    d_ones1 = nc.declare_dram_parameter("ones1", [1, 128], BF16, isOutput=False)
    d_sel2 = nc.declare_dram_parameter("sel2", [H, 6 * 128], BF16,
                                       isOutput=False)
    d_onesbd = nc.declare_dram_parameter("onesbd", [128, 2], BF16,
                                         isOutput=False)
    d_out = nc.declare_dram_parameter("out", [TOK, C], F32, isOutput=True)

    # 98-token tiles within a group (4 per group, batch-pair aligned)
    tts = [(o, 98) for o in range(0, T, 98)]

    with tile.TileContext(nc) as tc, \
         nc.allow_low_precision(reason="bf16 operands, fp32 accumulate"):
        with tc.tile_pool(name="wres", bufs=1) as wres, \
             tc.tile_pool(name="xT", bufs=2) as p_xT, \
             tc.tile_pool(name="qk", bufs=2) as p_qk, \
             tc.tile_pool(name="vscr", bufs=2) as p_vscr, \
             tc.tile_pool(name="eT", bufs=2) as p_eT, \
             tc.tile_pool(name="rr", bufs=2) as p_rr, \
             tc.tile_pool(name="unT", bufs=3) as p_unT, \
             tc.tile_pool(name="osb", bufs=2) as p_osb, \
             tc.tile_pool(name="psA", bufs=2, space="PSUM") as psA, \
             tc.tile_pool(name="psB", bufs=2, space="PSUM") as psB, \
             tc.tile_pool(name="psS", bufs=2, space="PSUM") as psS, \
             tc.tile_pool(name="psO", bufs=2, space="PSUM") as psO:

            # ---- resident weights / constants ----
            w_qkv = []
            for ci in range(6):
                t = wres.tile([128, 3 * C], BF16, tag=f"wqkv{ci}")
                nc.sync.dma_start(t[:], d_wqkv[128 * ci:128 * (ci + 1), :])
                w_qkv.append(t)
            w_proj = []
            for ci in range(6):
                t = wres.tile([128, C], BF16, tag=f"wproj{ci}")
                nc.sync.dma_start(t[:], d_wproj[128 * ci:128 * (ci + 1), :])
                w_proj.append(t)
            ones1 = wres.tile([1, 128], BF16, tag="ones1")
            nc.sync.dma_start(ones1[:], d_ones1[:])
            sel2 = wres.tile([H, 6 * 128], BF16, tag="sel2")
            nc.sync.dma_start(sel2[:], d_sel2[:])
            onesbd = wres.tile([128, 2], BF16, tag="onesbd")
            nc.sync.dma_start(onesbd[:], d_onesbd[:])
            bias_sb = wres.tile([1, C], BF16, tag="bias_sb")
            nc.sync.dma_start(bias_sb[:], d_bias[:])
            # double-buffered block-diagonal k/v, zeroed once
            kbds, vbds = [], []
            for s in range(2):
                kb = wres.tile([128, 6 * BDW], BF16, tag=f"kbd{s}")
                nc.vector.memset(kb[:], 0.0)
                kbds.append(kb)
                vb = wres.tile([128, 6 * BDW], BF16, tag=f"vbd{s}")
                nc.vector.memset(vb[:], 0.0)
                vbds.append(vb)

            state = {}

            def emit_qkv(g):
                """Phases B+C for group g: load xT, qk GEMM, v GEMM."""
                r0 = g * T
                kbd, vbd = kbds[g % 2], vbds[g % 2]
                xT = p_xT.tile([128, 6 * T], BF16, tag="xT", name="xT")
                xv = xT[:].rearrange("p (ci t) -> p ci t", t=T)
                nc.sync.dma_start(
                    xv,
                    d_xt[:].rearrange("(ci p) t -> p ci t", ci=6)
                        [:, :, r0:r0 + T])

                # q,k GEMM feature-major; q natural, k scattered block-diag
                qk = []
                for j in range(12):
                    pq = psA.tile([128, TP], F32, tag="psA", name="psa")
                    for ci in range(6):
                        nc.tensor.matmul(
                            pq[:, :T],
                            w_qkv[ci][:, 128 * j:128 * (j + 1)],
                            xv[:, ci, :],
                            start=(ci == 0), stop=(ci == 5))
                    if j < 6:
                        t = p_qk.tile([128, TP], BF16, tag=f"q{j}", name=f"q{j}")
                        nc.vector.tensor_copy(t[:, :T], pq[:, :T])
                        nc.vector.tensor_copy(t[:, T:T + 2], pq[:, :2])
                        qk.append(t)
                    else:
                        jj = j - 6
                        kv = kbd[:, jj * BDW:(jj + 1) * BDW].rearrange(
                            "p (b two t) -> p b two t", two=2, t=64)
                        nc.vector.tensor_copy(
                            kv[0:64, :, 0, 0:49],
                            pq[0:64, :T].rearrange("p (b t) -> p b t", t=49))
                        nc.vector.tensor_copy(
                            kv[64:128, :, 1, 0:49],
                            pq[64:128, :T].rearrange("p (b t) -> p b t", t=49))
                state[("qk", g)] = qk

                # v GEMM token-major over 98-token tiles + scatter
                v5 = vbd[:].rearrange("p (j b two c) -> p j b two c",
                                      b=G, two=2, c=64)
                for tti, (to, tk) in enumerate(tts):
                    scr = p_vscr.tile([98, C], BF16, tag="vscr", name="vscr")
                    for half in range(2):
                        pv = psB.tile([98, 384], F32, tag="psB", name="psb")
                        for ci in range(6):
                            nc.tensor.matmul(
                                pv[:, :],
                                xv[:, ci, to:to + tk],
                                w_qkv[ci][:, 1536 + 384 * half:
                                          1536 + 384 * (half + 1)],
                                start=(ci == 0), stop=(ci == 5))
                        nc.vector.tensor_copy(
                            scr[:, 384 * half:384 * (half + 1)], pv[:, :])
                    sv = scr[:].rearrange("p (j two c) -> p j two c",
                                          two=2, c=64)
                    for bl in range(2):
                        b = 2 * tti + bl
                        for two in range(2):
                            nc.sync.dma_start(
                                v5[64 * two:64 * two + 49, :, b, two, :],
                                sv[49 * bl:49 * bl + 49, :, two, :])

            def emit_attn_core(g):
                """Phase D core for group g: scores -> exp -> sums,
                unnormalized out^T. j-lookahead so the PE never waits on
                ACT exp: scores(j+1) is issued before r2/po(j)."""
                kbd, vbd = kbds[g % 2], vbds[g % 2]
                qk = state.pop(("qk", g))

                unT = [p_unT.tile([128, T], BF16, tag=f"unT{ci}",
                                  name=f"unT{ci}") for ci in range(6)]
                rgrp = p_rr.tile([H, T], F32, tag="rgrp", name="rgrp")
                eTs = {}

                def scores(j):
                    ps = psS.tile([128, 50 * G], F32, tag="psS", name="ps")
                    for b in range(G):
                        nc.tensor.matmul(
                            ps[:, 50 * b:50 * b + 50],
                            kbd[:, j * BDW + 128 * b:j * BDW + 128 * (b + 1)],
                            qk[j][:, 49 * b:49 * b + 50],
                            start=True, stop=True)
                    eT = p_eT.tile([128, 50 * G], BF16, tag="eT", name="eT")
                    nc.scalar.activation(eT[:], ps[:], EXP, scale=0.125)
                    eTs[j] = eT

                def finish(j):
                    eT = eTs.pop(j)
                    pr = psB.tile([2, 50 * G], F32, tag="psB", name="pr")
                    nc.tensor.matmul(pr[:], onesbd[:], eT[:],
                                     start=True, stop=True)
                    po = psO.tile([128, 50 * G], F32, tag="psO", name="po")
                    for b in range(G):
                        nc.tensor.matmul(
                            po[:, 50 * b:50 * b + 50],
                            vbd[:, j * BDW + 128 * b:j * BDW + 128 * (b + 1)],
                            eT[:, 50 * b:50 * b + 50],
                            start=True, stop=True)
                    nc.vector.tensor_copy(
                        unT[j][:, :].rearrange("p (b c) -> p b c", c=49),
                        po[:, :].rearrange("p (b c) -> p b c", c=50)[:, :, 0:49])
                    r2 = p_rr.tile([2, T], F32, tag="r2", name="r2")
                    nc.scalar.copy(
                        r2[:].rearrange("p (b c) -> p b c", c=49),
                        pr[:].rearrange("p (b c) -> p b c", c=50)[:, :, 0:49])
                    nc.sync.dma_start(rgrp[2 * j:2 * j + 2, :], r2[:])

                scores(0)
                for j in range(1, 6):
                    scores(j)
                    finish(j - 1)
                finish(5)
                state[("unT", g)] = unT
                state[("rgrp", g)] = rgrp

            def emit_norm_proj(g):
                """Phase D normalize + phase E proj for group g."""
                r0 = g * T
                unT = state.pop(("unT", g))
                rgrp = state.pop(("rgrp", g))
                rr32 = p_rr.tile([H, T], F32, tag="rr32", name="rr32")
                nc.vector.reciprocal_approx_fast(rr32[:], rgrp[:])
                rr = p_rr.tile([H, T], BF16, tag="rr", name="rr")
                nc.vector.tensor_copy(rr[:], rr32[:])
                for j in range(6):
                    pbc = psA.tile([128, TP], F32, tag="psA", name="psa")
                    nc.tensor.matmul(pbc[:, :T], sel2[:, 128 * j:128 * (j + 1)],
                                     rr[:], start=True, stop=True)
                    nc.vector.tensor_mul(out=unT[j][:, :], in0=unT[j][:, :],
                                         in1=pbc[:, :T])

                # proj GEMM + bias -> out
                for tti, (to, tk) in enumerate(tts):
                    osb = p_osb.tile([98, C], F32, tag="osb", name="osb")
                    for half in range(2):
                        pp = psB.tile([98, 384], F32, tag="psB", name="psb")
                        for ci in range(6):
                            nc.tensor.matmul(
                                pp[:, :],
                                unT[ci][:, to:to + tk],
                                w_proj[ci][:, 384 * half:384 * (half + 1)],
                                start=(ci == 0), stop=False)
                        nc.tensor.matmul(
                            pp[:, :], ones1[:, :tk],
                            bias_sb[:, 384 * half:384 * (half + 1)],
                            start=False, stop=True)
                        nc.vector.tensor_copy(
                            osb[:, 384 * half:384 * (half + 1)], pp[:, :])
                    nc.sync.dma_start(d_out[r0 + to:r0 + to + tk, :],
                                      osb[:, :])

            for step in range(N_GROUPS + 2):
                if step < N_GROUPS:
                    emit_qkv(step)
                if 1 <= step <= N_GROUPS:
                    emit_attn_core(step - 1)
                if step >= 2:
                    emit_norm_proj(step - 2)

    nc.compile()
    return nc


def make_in_maps(inputs):
    """Host-side prep: shard + transpose + cast. inputs keys as in
    reference.setup_inputs()."""
    import ml_dtypes
    bf16 = ml_dtypes.bfloat16

    x = np.asarray(inputs["x"], dtype=np.float32)
    B, N, Cc = x.shape
    W_qkv = np.ascontiguousarray(
        np.asarray(inputs["W_qkv"], dtype=np.float32).astype(bf16))
    W_proj = np.ascontiguousarray(
        np.asarray(inputs["W_proj"], dtype=np.float32).astype(bf16))
    bias = np.ascontiguousarray(
        np.asarray(inputs["b_proj"], dtype=np.float32)
        .reshape(1, Cc).astype(bf16))
    consts = _consts()

    in_maps = []
    for i in range(NUM_CORES):
        xt = np.ascontiguousarray(
            x[i * B_CORE:(i + 1) * B_CORE].reshape(TOK, Cc).T.astype(bf16))
        m = {"xt": xt, "wqkv": W_qkv, "wproj": W_proj, "bias": bias}
        m.update(consts)
        in_maps.append(m)
    return in_maps


def kernel(x, W_qkv, W_proj, b_proj):
    from concourse.bass_utils import run_bass_kernel_spmd

    if "nc" not in _CACHE:
        _CACHE["nc"] = _build()
    nc = _CACHE["nc"]

    in_maps = make_in_maps(
        {"x": x, "W_qkv": W_qkv, "W_proj": W_proj, "b_proj": b_proj})
    res = run_bass_kernel_spmd(nc, in_maps, list(range(NUM_CORES)))
    B, N, Cc = np.asarray(x).shape
    out = np.empty((B, N, Cc), dtype=np.float32)
    for i in range(NUM_CORES):
        out[i * B_CORE:(i + 1) * B_CORE] = res.results[i]["out"].reshape(
            B_CORE, N, Cc)
    return out


# revision 14
# speedup vs baseline: 2.3929x; 1.1920x over previous
"""Trainium2 Bass kernel: fused attention block (QKV proj -> MHA -> out proj).

Reference (per batch item b, NUM_HEADS=12, Dh=64):
    qkv = x @ W_qkv; q,k,v per head
    attn = softmax(q @ k^T / 8) @ v
    out  = concat_heads(attn) @ W_proj + b_proj

Sharding: data-parallel over batch across 8 NeuronCores (128 batch items
per core), weights replicated. One SPMD Bass program, per-core inputs.

Per-core plan (128 batches, groups of G=8 batches = 392 tokens). All
matmul operands bf16 (fp32 accumulate in PSUM). x is pre-transposed +
cast on host to feature-major xT[768, 6272] per core, so no on-device
transposes. Software-pipelined one group deep: attention+proj of group
g-1 is emitted alongside QKV of group g so the PE never idles.

  B. q,k GEMM feature-major: psum[128co, T] = Wqkv_tile.T @ xT.
     q co-tiles stored naturally [128, T+2]; k co-tiles DVE-scattered
     into block-diagonal kbd[g%2]: per (j, b) a [128, 128] block with
     k_h(2j) rows 0:64 cols 0:49 and k_h(2j+1) rows 64:128 cols 64:113
     (zeros elsewhere, memset once).
  C. v GEMM token-major over 98-token tiles -> scratch; 4 aligned
     SBUF->SBUF DMAs per tile scatter into vbd[g%2]: per (j, b) a
     [128, 128] block with v_h(2j) rows 0:49 cols 0:64 and v_h(2j+1)
     rows 64:113 cols 64:128.
  D. attention per (head-pair j, batch): both heads in one matmul chain:
       sT2 = kbd_jb.T @ q_pair        [128, 50]  scores, heads stacked
       eT2 = exp(sT2/8)               ACT, [128, 400] per j
       r2  = onesbd.T @ eT2           [2, 400]   row sums per head
       po  = vbd_jb.T @ eT2           [128, 50]  unnorm out^T, both heads
     r rows gathered (ACT copy + SBUF-shift DMA) into rgrp[12, T];
     one batched reciprocal per group; per j: broadcast matmul
     (sel2 selector, f32r) -> psum bc[128, T]; unT[j] *= bc (DVE).
  E. proj GEMM token-major: psum[98tok, 384] = unT_slice.T @ Wproj,
     bias accumulated via ones-row matmul -> copy -> DMA out.
"""
import sys

sys.path.insert(0, "/opt/trn_rl_repo")

import numpy as np

NUM_CORES = 8
B_CORE = 128          # batch items per core
SEQ = 49              # tokens per batch item
C = 768               # channels
H = 12                # heads
G = 8                 # batch items per group
T = SEQ * G           # 392 tokens per group (even)
TP = T + 2            # padded q tile width
TOK = B_CORE * SEQ    # 6272 tokens per core
N_GROUPS = B_CORE // G
BDW = G * 128         # kbd/vbd block row width per j (padded 128/batch)

_CACHE = {}


def _consts():
    import ml_dtypes
    bf16 = ml_dtypes.bfloat16
    # sel2: broadcast head sums over Dh partitions (bf16 matmul)
    sel2 = np.zeros((H, 6 * 128), dtype=bf16)
    for j in range(6):
        sel2[2 * j, 128 * j:128 * j + 64] = 1.0
        sel2[2 * j + 1, 128 * j + 64:128 * (j + 1)] = 1.0
    # onesbd: column sums of eT rows per head (keys at rows 0:49, 64:113)
    onesbd = np.zeros((128, 2), dtype=bf16)
    onesbd[0:49, 0] = 1.0
    onesbd[64:113, 1] = 1.0
    return {"sel2": sel2, "onesbd": onesbd}


def _build():
    import concourse.bacc as bacc
    import concourse.mybir as mybir
    import concourse.tile as tile

    F32 = mybir.dt.float32
    F32R = mybir.dt.float32r
    BF16 = mybir.dt.bfloat16
    EXP = mybir.ActivationFunctionType.Exp

    nc = bacc.Bacc("TRN2", target_bir_lowering=False)

    d_xt = nc.declare_dram_parameter("xt", [C, TOK + 2], BF16, isOutput=False)
    d_wqkv = nc.declare_dram_parameter("wqkv", [C, 3 * C], BF16, isOutput=False)
    d_wproj = nc.declare_dram_parameter("wproj", [C, C], BF16, isOutput=False)
    d_biast = nc.declare_dram_parameter("biast", [128, 6], BF16,
                                        isOutput=False)
    d_sel2 = nc.declare_dram_parameter("sel2", [H, 6 * 128], BF16,
                                       isOutput=False)
    d_onesbd = nc.declare_dram_parameter("onesbd", [128, 2], BF16,
                                         isOutput=False)
    d_out = nc.declare_dram_parameter("out", [C, TOK], F32, isOutput=True)

    # 98-token tiles within a group (4 per group, batch-pair aligned)
    tts = [(o, 98) for o in range(0, T, 98)]

    with tile.TileContext(nc) as tc, \
         nc.allow_low_precision(reason="bf16 operands, fp32 accumulate"):
        with tc.tile_pool(name="wres", bufs=1) as wres, \
             tc.tile_pool(name="xT", bufs=2) as p_xT, \
             tc.tile_pool(name="qk", bufs=2) as p_qk, \
             tc.tile_pool(name="vscr", bufs=2) as p_vscr, \
             tc.tile_pool(name="eT", bufs=2) as p_eT, \
             tc.tile_pool(name="rr", bufs=2) as p_rr, \
             tc.tile_pool(name="unT", bufs=3) as p_unT, \
             tc.tile_pool(name="osb", bufs=2) as p_osb, \
             tc.tile_pool(name="psA", bufs=2, space="PSUM") as psA, \
             tc.tile_pool(name="psB", bufs=2, space="PSUM") as psB, \
             tc.tile_pool(name="psS", bufs=2, space="PSUM") as psS, \
             tc.tile_pool(name="psO", bufs=2, space="PSUM") as psO:

            state = {}

            def issue_x_dma(g):
                xT = p_xT.tile([128, 6 * TP], BF16, tag="xT", name="xT")
                xv = xT[:].rearrange("p (ci t) -> p ci t", t=TP)
                nc.sync.dma_start(
                    xv,
                    d_xt[:].rearrange("(ci p) t -> p ci t", ci=6)
                        [:, :, g * T:g * T + TP])
                state[("xv", g)] = xv

            # group 0's x first so the PE can start ASAP
            issue_x_dma(0)

            # ---- resident weights / constants ----
            w_qkv = []
            for ci in range(6):
                t = wres.tile([128, 3 * C], BF16, tag=f"wqkv{ci}")
                nc.sync.dma_start(t[:], d_wqkv[128 * ci:128 * (ci + 1), :])
                w_qkv.append(t)
            w_proj = []
            for ci in range(6):
                t = wres.tile([128, C], BF16, tag=f"wproj{ci}")
                nc.sync.dma_start(t[:], d_wproj[128 * ci:128 * (ci + 1), :])
                w_proj.append(t)
            sel2 = wres.tile([H, 6 * 128], BF16, tag="sel2")
            nc.sync.dma_start(sel2[:], d_sel2[:])
            onesbd = wres.tile([128, 2], BF16, tag="onesbd")
            nc.sync.dma_start(onesbd[:], d_onesbd[:])
            biast = wres.tile([128, 6], BF16, tag="biast")
            nc.sync.dma_start(biast[:], d_biast[:])
            # double-buffered block-diagonal k/v, zeroed once
            kbds, vbds = [], []
            for s in range(2):
                kb = wres.tile([128, 6 * BDW], BF16, tag=f"kbd{s}")
                nc.vector.memset(kb[:], 0.0)
                kbds.append(kb)
                vb = wres.tile([128, 6 * BDW], BF16, tag=f"vbd{s}")
                nc.vector.memset(vb[:], 0.0)
                vbds.append(vb)

            def emit_qkv(g):
                """Phases B+C for group g: load xT, qk GEMM, v GEMM."""
                kbd, vbd = kbds[g % 2], vbds[g % 2]
                if ("xv", g) not in state:
                    issue_x_dma(g)
                xv = state.pop(("xv", g))

                # q,k GEMM feature-major; q natural, k scattered block-diag
                qk = []
                for j in range(12):
                    pq = psA.tile([128, TP], F32, tag="psA", name="psa")
                    for ci in range(6):
                        nc.tensor.matmul(
                            pq[:, :TP],
                            w_qkv[ci][:, 128 * j:128 * (j + 1)],
                            xv[:, ci, :],
                            start=(ci == 0), stop=(ci == 5))
                    if j < 6:
                        t = p_qk.tile([128, TP], BF16, tag=f"q{j}", name=f"q{j}")
                        nc.vector.tensor_copy(t[:], pq[:])
                        qk.append(t)
                    else:
                        jj = j - 6
                        kv = kbd[:, jj * BDW:(jj + 1) * BDW].rearrange(
                            "p (b two t) -> p b two t", two=2, t=64)
                        nc.scalar.copy(
                            kv[0:64, :, 0, 0:49],
                            pq[0:64, :T].rearrange("p (b t) -> p b t", t=49))
                        nc.scalar.copy(
                            kv[64:128, :, 1, 0:49],
                            pq[64:128, :T].rearrange("p (b t) -> p b t", t=49))
                state[("qk", g)] = qk

                # v GEMM token-major over 98-token tiles + scatter
                v5 = vbd[:].rearrange("p (j b two c) -> p j b two c",
                                      b=G, two=2, c=64)
                for tti, (to, tk) in enumerate(tts):
                    scr = p_vscr.tile([98, C], BF16, tag="vscr", name="vscr")
                    for half in range(2):
                        pv = psB.tile([98, 384], F32, tag="psB", name="psb")
                        for ci in range(6):
                            nc.tensor.matmul(
                                pv[:, :],
                                xv[:, ci, to:to + tk],
                                w_qkv[ci][:, 1536 + 384 * half:
                                          1536 + 384 * (half + 1)],
                                start=(ci == 0), stop=(ci == 5))
                        nc.vector.tensor_copy(
                            scr[:, 384 * half:384 * (half + 1)], pv[:, :])
                    sv = scr[:].rearrange("p (j two c) -> p j two c",
                                          two=2, c=64)
                    for bl in range(2):
                        b = 2 * tti + bl
                        for two in range(2):
                            nc.sync.dma_start(
                                v5[64 * two:64 * two + 49, :, b, two, :],
                                sv[49 * bl:49 * bl + 49, :, two, :])

            def emit_attn_core(g):
                """Phase D core for group g: scores -> exp -> sums,
                unnormalized out^T. j-lookahead so the PE never waits on
                ACT exp: scores(j+1) is issued before r2/po(j)."""
                kbd, vbd = kbds[g % 2], vbds[g % 2]
                qk = state.pop(("qk", g))

                unT = [p_unT.tile([128, T], BF16, tag=f"unT{ci}",
                                  name=f"unT{ci}") for ci in range(6)]
                rgrp = p_rr.tile([H, T], F32, tag="rgrp", name="rgrp")
                eTs = {}

                def scores(j):
                    ps = psS.tile([128, 50 * G], F32, tag="psS", name="ps")
                    for b in range(G):
                        nc.tensor.matmul(
                            ps[:, 50 * b:50 * b + 50],
                            kbd[:, j * BDW + 128 * b:j * BDW + 128 * (b + 1)],
                            qk[j][:, 49 * b:49 * b + 50],
                            start=True, stop=True)
                    eT = p_eT.tile([128, 50 * G], BF16, tag="eT", name="eT")
                    nc.scalar.activation(eT[:], ps[:], EXP, scale=0.125)
                    eTs[j] = eT

                def finish(j):
                    eT = eTs.pop(j)
                    pr = psO.tile([2, 50 * G], F32, tag="psO", name="pr")
                    nc.tensor.matmul(pr[:], onesbd[:], eT[:],
                                     start=True, stop=True)
                    po = psO.tile([128, 50 * G], F32, tag="psO", name="po")
                    for b in range(G):
                        nc.tensor.matmul(
                            po[:, 50 * b:50 * b + 50],
                            vbd[:, j * BDW + 128 * b:j * BDW + 128 * (b + 1)],
                            eT[:, 50 * b:50 * b + 50],
                            start=True, stop=True)
                    nc.vector.tensor_copy(
                        unT[j][:, :].rearrange("p (b c) -> p b c", c=49),
                        po[:, :].rearrange("p (b c) -> p b c", c=50)[:, :, 0:49])
                    r2 = p_rr.tile([2, T], F32, tag="r2", name="r2")
                    nc.scalar.copy(
                        r2[:].rearrange("p (b c) -> p b c", c=49),
                        pr[:].rearrange("p (b c) -> p b c", c=50)[:, :, 0:49])
                    nc.sync.dma_start(rgrp[2 * j:2 * j + 2, :], r2[:])

                scores(0)
                for j in range(1, 6):
                    scores(j)
                    finish(j - 1)
                finish(5)
                state[("unT", g)] = unT
                state[("rgrp", g)] = rgrp

            def emit_norm_proj(g):
                """Phase D normalize + phase E proj for group g."""
                r0 = g * T
                unT = state.pop(("unT", g))
                rgrp = state.pop(("rgrp", g))
                rr32 = p_rr.tile([H, T], F32, tag="rr32", name="rr32")
                nc.vector.reciprocal_approx_fast(rr32[:], rgrp[:])
                rr = p_rr.tile([H, T], BF16, tag="rr", name="rr")
                nc.vector.tensor_copy(rr[:], rr32[:])
                for j in range(6):
                    pbc = psA.tile([128, TP], F32, tag="psA", name="psa")
                    nc.tensor.matmul(pbc[:, :T], sel2[:, 128 * j:128 * (j + 1)],
                                     rr[:], start=True, stop=True)
                    nc.vector.tensor_mul(out=unT[j][:, :], in0=unT[j][:, :],
                                         in1=pbc[:, :T])

                # proj GEMM feature-major + bias broadcast -> outT
                for co in range(6):
                    pp = psA.tile([128, TP], F32, tag="psA", name="psa")
                    for ci in range(6):
                        nc.tensor.matmul(
                            pp[:, :T],
                            w_proj[ci][:, 128 * co:128 * (co + 1)],
                            unT[ci][:, :],
                            start=(ci == 0), stop=(ci == 5))
                    osb = p_osb.tile([128, T], F32, tag="osb", name="osb")
                    nc.vector.tensor_add(
                        out=osb[:], in0=pp[:, :T],
                        in1=biast[:, co:co + 1].broadcast_to([128, T]))
                    nc.sync.dma_start(
                        d_out[128 * co:128 * (co + 1), r0:r0 + T], osb[:])

            for step in range(N_GROUPS + 2):
                if step < N_GROUPS:
                    emit_qkv(step)
                if 1 <= step <= N_GROUPS:
                    emit_attn_core(step - 1)
                if step >= 2:
                    emit_norm_proj(step - 2)

    nc.compile()
    return nc


def make_in_maps(inputs):
    """Host-side prep: shard + transpose + cast. inputs keys as in
    reference.setup_inputs()."""
    import ml_dtypes
    bf16 = ml_dtypes.bfloat16

    x = np.asarray(inputs["x"], dtype=np.float32)
    B, N, Cc = x.shape
    W_qkv = np.ascontiguousarray(
        np.asarray(inputs["W_qkv"], dtype=np.float32).astype(bf16))
    W_proj = np.ascontiguousarray(
        np.asarray(inputs["W_proj"], dtype=np.float32).astype(bf16))
    biast = np.ascontiguousarray(
        np.asarray(inputs["b_proj"], dtype=np.float32)
        .reshape(6, 128).T.astype(bf16))
    consts = _consts()

    in_maps = []
    for i in range(NUM_CORES):
        xt = np.zeros((Cc, TOK + 2), dtype=bf16)
        xt[:, :TOK] = (
            x[i * B_CORE:(i + 1) * B_CORE].reshape(TOK, Cc).T.astype(bf16))
        m = {"xt": xt, "wqkv": W_qkv, "wproj": W_proj, "biast": biast}
        m.update(consts)
        in_maps.append(m)
    return in_maps


def kernel(x, W_qkv, W_proj, b_proj):
    from concourse.bass_utils import run_bass_kernel_spmd

    if "nc" not in _CACHE:
        _CACHE["nc"] = _build()
    nc = _CACHE["nc"]

    in_maps = make_in_maps(
        {"x": x, "W_qkv": W_qkv, "W_proj": W_proj, "b_proj": b_proj})
    res = run_bass_kernel_spmd(nc, in_maps, list(range(NUM_CORES)))
    B, N, Cc = np.asarray(x).shape
    out = np.empty((B, N, Cc), dtype=np.float32)
    for i in range(NUM_CORES):
        out[i * B_CORE:(i + 1) * B_CORE] = res.results[i]["out"].T.reshape(
            B_CORE, N, Cc)
    return out


# revision 16
# speedup vs baseline: 2.3954x; 1.0010x over previous
"""Trainium2 Bass kernel: fused attention block (QKV proj -> MHA -> out proj).

Reference (per batch item b, NUM_HEADS=12, Dh=64):
    qkv = x @ W_qkv; q,k,v per head
    attn = softmax(q @ k^T / 8) @ v
    out  = concat_heads(attn) @ W_proj + b_proj

Sharding: data-parallel over batch across 8 NeuronCores (128 batch items
per core), weights replicated. One SPMD Bass program, per-core inputs.

Per-core plan (128 batches, groups of G=8 batches = 392 tokens). All
matmul operands bf16 (fp32 accumulate in PSUM). x is pre-transposed +
cast on host to feature-major xT[768, 6272] per core, so no on-device
transposes. Software-pipelined one group deep: attention+proj of group
g-1 is emitted alongside QKV of group g so the PE never idles.

  B. q,k GEMM feature-major: psum[128co, T] = Wqkv_tile.T @ xT.
     q co-tiles stored naturally [128, T+2]; k co-tiles DVE-scattered
     into block-diagonal kbd[g%2]: per (j, b) a [128, 128] block with
     k_h(2j) rows 0:64 cols 0:49 and k_h(2j+1) rows 64:128 cols 64:113
     (zeros elsewhere, memset once).
  C. v GEMM token-major over 98-token tiles -> scratch; 4 aligned
     SBUF->SBUF DMAs per tile scatter into vbd[g%2]: per (j, b) a
     [128, 128] block with v_h(2j) rows 0:49 cols 0:64 and v_h(2j+1)
     rows 64:113 cols 64:128.
  D. attention per (head-pair j, batch): both heads in one matmul chain:
       sT2 = kbd_jb.T @ q_pair        [128, 50]  scores, heads stacked
       eT2 = exp(sT2/8)               ACT, [128, 400] per j
       r2  = onesbd.T @ eT2           [2, 400]   row sums per head
       po  = vbd_jb.T @ eT2           [128, 50]  unnorm out^T, both heads
     r rows gathered (ACT copy + SBUF-shift DMA) into rgrp[12, T];
     one batched reciprocal per group; per j: broadcast matmul
     (sel2 selector, f32r) -> psum bc[128, T]; unT[j] *= bc (DVE).
  E. proj GEMM token-major: psum[98tok, 384] = unT_slice.T @ Wproj,
     bias accumulated via ones-row matmul -> copy -> DMA out.
"""
import sys

sys.path.insert(0, "/opt/trn_rl_repo")

import numpy as np

NUM_CORES = 8
B_CORE = 128          # batch items per core
SEQ = 49              # tokens per batch item
C = 768               # channels
H = 12                # heads
G = 8                 # batch items per group
T = SEQ * G           # 392 tokens per group (even)
TP = T + 2            # padded q tile width
TOK = B_CORE * SEQ    # 6272 tokens per core
N_GROUPS = B_CORE // G
BDW = G * 128         # kbd/vbd block row width per j (padded 128/batch)

_CACHE = {}


def _consts():
    import ml_dtypes
    bf16 = ml_dtypes.bfloat16
    # sel2: broadcast head sums over Dh partitions (bf16 matmul)
    sel2 = np.zeros((H, 6 * 128), dtype=bf16)
    for j in range(6):
        sel2[2 * j, 128 * j:128 * j + 64] = 1.0
        sel2[2 * j + 1, 128 * j + 64:128 * (j + 1)] = 1.0
    # onesbd: column sums of eT rows per head (keys at rows 0:49, 64:113)
    onesbd = np.zeros((128, 2), dtype=bf16)
    onesbd[0:49, 0] = 1.0
    onesbd[64:113, 1] = 1.0
    return {"sel2": sel2, "onesbd": onesbd}


def _build():
    import concourse.bacc as bacc
    import concourse.mybir as mybir
    import concourse.tile as tile

    F32 = mybir.dt.float32
    F32R = mybir.dt.float32r
    BF16 = mybir.dt.bfloat16
    EXP = mybir.ActivationFunctionType.Exp

    nc = bacc.Bacc("TRN2", target_bir_lowering=False)

    d_xt = nc.declare_dram_parameter("xt", [C, TOK + 2], BF16, isOutput=False)
    d_wqkv = nc.declare_dram_parameter("wqkv", [C, 3 * C], BF16, isOutput=False)
    d_wproj = nc.declare_dram_parameter("wproj", [C, C], BF16, isOutput=False)
    d_biast = nc.declare_dram_parameter("biast", [128, 6], BF16,
                                        isOutput=False)
    d_sel2 = nc.declare_dram_parameter("sel2", [H, 6 * 128], BF16,
                                       isOutput=False)
    d_onesbd = nc.declare_dram_parameter("onesbd", [128, 2], BF16,
                                         isOutput=False)
    d_out = nc.declare_dram_parameter("out", [C, TOK], F32, isOutput=True)

    # 98-token tiles within a group (4 per group, batch-pair aligned)
    tts = [(o, 98) for o in range(0, T, 98)]

    with tile.TileContext(nc) as tc, \
         nc.allow_low_precision(reason="bf16 operands, fp32 accumulate"):
        with tc.tile_pool(name="wres", bufs=1) as wres, \
             tc.tile_pool(name="xT", bufs=2) as p_xT, \
             tc.tile_pool(name="qk", bufs=2) as p_qk, \
             tc.tile_pool(name="vscr", bufs=2) as p_vscr, \
             tc.tile_pool(name="eT", bufs=2) as p_eT, \
             tc.tile_pool(name="rr", bufs=2) as p_rr, \
             tc.tile_pool(name="unT", bufs=3) as p_unT, \
             tc.tile_pool(name="osb", bufs=2) as p_osb, \
             tc.tile_pool(name="psA", bufs=2, space="PSUM") as psA, \
             tc.tile_pool(name="psB", bufs=2, space="PSUM") as psB, \
             tc.tile_pool(name="psS", bufs=2, space="PSUM") as psS, \
             tc.tile_pool(name="psO", bufs=2, space="PSUM") as psO:

            state = {}

            def issue_x_dma(g):
                xT = p_xT.tile([128, 6 * TP], BF16, tag="xT", name="xT")
                xv = xT[:].rearrange("p (ci t) -> p ci t", t=TP)
                nc.sync.dma_start(
                    xv,
                    d_xt[:].rearrange("(ci p) t -> p ci t", ci=6)
                        [:, :, g * T:g * T + TP])
                state[("xv", g)] = xv

            # group 0's x first so the PE can start ASAP
            issue_x_dma(0)

            # ---- resident weights / constants ----
            w_qkv = []
            for ci in range(6):
                t = wres.tile([128, 3 * C], BF16, tag=f"wqkv{ci}")
                nc.sync.dma_start(t[:], d_wqkv[128 * ci:128 * (ci + 1), :])
                w_qkv.append(t)
            w_proj = []
            for ci in range(6):
                t = wres.tile([128, C], BF16, tag=f"wproj{ci}")
                nc.sync.dma_start(t[:], d_wproj[128 * ci:128 * (ci + 1), :])
                w_proj.append(t)
            sel2 = wres.tile([H, 6 * 128], BF16, tag="sel2")
            nc.sync.dma_start(sel2[:], d_sel2[:])
            onesbd = wres.tile([128, 2], BF16, tag="onesbd")
            nc.sync.dma_start(onesbd[:], d_onesbd[:])
            biast = wres.tile([128, 6], BF16, tag="biast")
            nc.sync.dma_start(biast[:], d_biast[:])
            # double-buffered block-diagonal k/v, zeroed once
            kbds, vbds = [], []
            for s in range(2):
                kb = wres.tile([128, 6 * BDW], BF16, tag=f"kbd{s}")
                nc.vector.memset(kb[:], 0.0)
                kbds.append(kb)
                vb = wres.tile([128, 6 * BDW], BF16, tag=f"vbd{s}")
                nc.vector.memset(vb[:], 0.0)
                vbds.append(vb)

            def qk_cotile(g, j):
                """One qkv co-tile: 6 MMs; q copy or k block-diag scatter."""
                kbd = kbds[g % 2]
                xv = state[("xv", g)]
                pq = psA.tile([128, TP], F32, tag="psA", name="psa")
                for ci in range(6):
                    nc.tensor.matmul(
                        pq[:, :TP],
                        w_qkv[ci][:, 128 * j:128 * (j + 1)],
                        xv[:, ci, :],
                        start=(ci == 0), stop=(ci == 5))
                if j < 6:
                    t = p_qk.tile([128, TP], BF16, tag=f"q{j}", name=f"q{j}")
                    nc.vector.tensor_copy(t[:], pq[:])
                    state[("q", g, j)] = t
                else:
                    jj = j - 6
                    kv = kbd[:, jj * BDW:(jj + 1) * BDW].rearrange(
                        "p (b two t) -> p b two t", two=2, t=64)
                    nc.scalar.copy(
                        kv[0:64, :, 0, 0:49],
                        pq[0:64, :T].rearrange("p (b t) -> p b t", t=49))
                    nc.scalar.copy(
                        kv[64:128, :, 1, 0:49],
                        pq[64:128, :T].rearrange("p (b t) -> p b t", t=49))

            def v_tile(g, tti):
                """One 98-token v tile: 12 MMs + 2 copies + 4 scatter DMAs."""
                vbd = vbds[g % 2]
                xv = state[("xv", g)]
                v5 = vbd[:].rearrange("p (j b two c) -> p j b two c",
                                      b=G, two=2, c=64)
                to, tk = tts[tti]
                scr = p_vscr.tile([98, C], BF16, tag="vscr", name="vscr")
                for half in range(2):
                    pv = psB.tile([98, 384], F32, tag="psB", name="psb")
                    for ci in range(6):
                        nc.tensor.matmul(
                            pv[:, :],
                            xv[:, ci, to:to + tk],
                            w_qkv[ci][:, 1536 + 384 * half:
                                      1536 + 384 * (half + 1)],
                            start=(ci == 0), stop=(ci == 5))
                    nc.vector.tensor_copy(
                        scr[:, 384 * half:384 * (half + 1)], pv[:, :])
                sv = scr[:].rearrange("p (j two c) -> p j two c",
                                      two=2, c=64)
                for bl in range(2):
                    b = 2 * tti + bl
                    for two in range(2):
                        nc.sync.dma_start(
                            v5[64 * two:64 * two + 49, :, b, two, :],
                            sv[49 * bl:49 * bl + 49, :, two, :])

            def attn_sc(g, j):
                """Scores + exp for head-pair j of group g."""
                kbd = kbds[g % 2]
                if j == 0:
                    state[("unT", g)] = [
                        p_unT.tile([128, T], BF16, tag=f"unT{ci}",
                                   name=f"unT{ci}") for ci in range(6)]
                    state[("rgrp", g)] = p_rr.tile([H, T], F32, tag="rgrp",
                                                   name="rgrp")
                qj = state.pop(("q", g, j))
                ps = psS.tile([128, 50 * G], F32, tag="psS", name="ps")
                for b in range(G):
                    nc.tensor.matmul(
                        ps[:, 50 * b:50 * b + 50],
                        kbd[:, j * BDW + 128 * b:j * BDW + 128 * (b + 1)],
                        qj[:, 49 * b:49 * b + 50],
                        start=True, stop=True)
                eT = p_eT.tile([128, 50 * G], BF16, tag="eT", name="eT")
                nc.scalar.activation(eT[:], ps[:], EXP, scale=0.125)
                state[("eT", g, j)] = eT

            def attn_fin(g, j):
                """Row sums + po + unT extraction for head-pair j."""
                vbd = vbds[g % 2]
                eT = state.pop(("eT", g, j))
                unT = state[("unT", g)]
                rgrp = state[("rgrp", g)]
                pr = psO.tile([2, 50 * G], F32, tag="psO", name="pr")
                nc.tensor.matmul(pr[:], onesbd[:], eT[:],
                                 start=True, stop=True)
                po = psO.tile([128, 50 * G], F32, tag="psO", name="po")
                for b in range(G):
                    nc.tensor.matmul(
                        po[:, 50 * b:50 * b + 50],
                        vbd[:, j * BDW + 128 * b:j * BDW + 128 * (b + 1)],
                        eT[:, 50 * b:50 * b + 50],
                        start=True, stop=True)
                nc.vector.tensor_copy(
                    unT[j][:, :].rearrange("p (b c) -> p b c", c=49),
                    po[:, :].rearrange("p (b c) -> p b c", c=50)[:, :, 0:49])
                r2 = p_rr.tile([2, T], F32, tag="r2", name="r2")
                nc.scalar.copy(
                    r2[:].rearrange("p (b c) -> p b c", c=49),
                    pr[:].rearrange("p (b c) -> p b c", c=50)[:, :, 0:49])
                nc.sync.dma_start(rgrp[2 * j:2 * j + 2, :], r2[:])

            def recip(g):
                rgrp = state.pop(("rgrp", g))
                rr32 = p_rr.tile([H, T], F32, tag="rr32", name="rr32")
                nc.vector.reciprocal_approx_fast(rr32[:], rgrp[:])
                rr = p_rr.tile([H, T], BF16, tag="rr", name="rr")
                nc.vector.tensor_copy(rr[:], rr32[:])
                state[("rr", g)] = rr

            def bcmul(g, j):
                """Broadcast 1/r over Dh partitions; normalize unT[j]."""
                rr = state[("rr", g)]
                unT = state[("unT", g)]
                pbc = psA.tile([128, TP], F32, tag="psA", name="psa")
                nc.tensor.matmul(pbc[:, :T], sel2[:, 128 * j:128 * (j + 1)],
                                 rr[:], start=True, stop=True)
                nc.vector.tensor_mul(out=unT[j][:, :], in0=unT[j][:, :],
                                     in1=pbc[:, :T])
                if j == 5:
                    del state[("rr", g)]

            def proj_co(g, co):
                """One out co-tile: 6 MMs + bias add + DMA out."""
                r0 = g * T
                unT = state[("unT", g)]
                pp = psA.tile([128, TP], F32, tag="psA", name="psa")
                for ci in range(6):
                    nc.tensor.matmul(
                        pp[:, :T],
                        w_proj[ci][:, 128 * co:128 * (co + 1)],
                        unT[ci][:, :],
                        start=(ci == 0), stop=(ci == 5))
                osb = p_osb.tile([128, T], F32, tag="osb", name="osb")
                nc.vector.tensor_add(
                    out=osb[:], in0=pp[:, :T],
                    in1=biast[:, co:co + 1].broadcast_to([128, T]))
                nc.sync.dma_start(
                    d_out[128 * co:128 * (co + 1), r0:r0 + T], osb[:])
                if co == 5:
                    del state[("unT", g)]

            issue_x_dma(1)

            # Per-step schedule: A = qkv group, B = attention group,
            # C = normalize+proj group. Attention/bc cells are interleaved
            # between big-GEMM chunks so their LDWEIGHTS hide under long
            # matmuls and ACT/DVE latencies never stall the PE.
            STEP = [
                ("recip", "C", 0),
                ("qk", "A", 0), ("sc", "B", 0), ("qk", "A", 1),
                ("bc", "C", 0),
                ("qk", "A", 2), ("sc", "B", 1), ("qk", "A", 3),
                ("bc", "C", 1),
                ("qk", "A", 4), ("fin", "B", 0), ("qk", "A", 5),
                ("bc", "C", 2),
                ("qk", "A", 6), ("sc", "B", 2), ("qk", "A", 7),
                ("bc", "C", 3),
                ("qk", "A", 8), ("fin", "B", 1), ("qk", "A", 9),
                ("bc", "C", 4),
                ("qk", "A", 10), ("sc", "B", 3), ("qk", "A", 11),
                ("bc", "C", 5), ("xdma", "A", 1),
                ("v", "A", 0), ("fin", "B", 2), ("proj", "C", 0),
                ("sc", "B", 4),
                ("v", "A", 1), ("proj", "C", 1), ("fin", "B", 3),
                ("sc", "B", 5),
                ("v", "A", 2), ("proj", "C", 2), ("fin", "B", 4),
                ("v", "A", 3), ("proj", "C", 3), ("fin", "B", 5),
                ("proj", "C", 4), ("proj", "C", 5),
            ]
            FN = {"qk": qk_cotile, "v": v_tile, "sc": attn_sc,
                  "fin": attn_fin, "bc": bcmul, "proj": proj_co}
            for step in range(N_GROUPS + 2):
                A, B_, C_ = step, step - 1, step - 2
                for item in STEP:
                    op, grp, idx = item
                    g = {"A": A, "B": B_, "C": C_}[grp]
                    if op == "xdma":
                        t = A + 1
                        if 0 <= t < N_GROUPS and ("xv", t) not in state:
                            issue_x_dma(t)
                        continue
                    if op == "recip":
                        if 0 <= g < N_GROUPS:
                            recip(g)
                        continue
                    if op in ("qk", "v") and not (0 <= g < N_GROUPS):
                        continue
                    if op in ("sc", "fin") and not (0 <= g < N_GROUPS):
                        continue
                    if op in ("bc", "proj") and not (0 <= g < N_GROUPS):
                        continue
                    FN[op](g, idx)

    nc.compile()
    return nc


def make_in_maps(inputs):
    """Host-side prep: shard + transpose + cast. inputs keys as in
    reference.setup_inputs()."""
    import ml_dtypes
    bf16 = ml_dtypes.bfloat16

    x = np.asarray(inputs["x"], dtype=np.float32)
    B, N, Cc = x.shape
    W_qkv = np.ascontiguousarray(
        np.asarray(inputs["W_qkv"], dtype=np.float32).astype(bf16))
    W_proj = np.ascontiguousarray(
        np.asarray(inputs["W_proj"], dtype=np.float32).astype(bf16))
    biast = np.ascontiguousarray(
        np.asarray(inputs["b_proj"], dtype=np.float32)
        .reshape(6, 128).T.astype(bf16))
    consts = _consts()

    in_maps = []
    for i in range(NUM_CORES):
        xt = np.zeros((Cc, TOK + 2), dtype=bf16)
        xt[:, :TOK] = (
            x[i * B_CORE:(i + 1) * B_CORE].reshape(TOK, Cc).T.astype(bf16))
        m = {"xt": xt, "wqkv": W_qkv, "wproj": W_proj, "biast": biast}
        m.update(consts)
        in_maps.append(m)
    return in_maps


def kernel(x, W_qkv, W_proj, b_proj):
    from concourse.bass_utils import run_bass_kernel_spmd

    if "nc" not in _CACHE:
        _CACHE["nc"] = _build()
    nc = _CACHE["nc"]

    in_maps = make_in_maps(
        {"x": x, "W_qkv": W_qkv, "W_proj": W_proj, "b_proj": b_proj})
    res = run_bass_kernel_spmd(nc, in_maps, list(range(NUM_CORES)))
    B, N, Cc = np.asarray(x).shape
    out = np.empty((B, N, Cc), dtype=np.float32)
    for i in range(NUM_CORES):
        out[i * B_CORE:(i + 1) * B_CORE] = res.results[i]["out"].T.reshape(
            B_CORE, N, Cc)
    return out
